# revision 30
# baseline (speedup 1.0000x reference)
"""Trainium2 Bass kernel for nn_Attention (additive-attention scoring module).

reference math (B=128, S=1024, D=512):
    qp[i,o]   = sum_d q[i,d] Wq[o,d] + bq[o]
    e[i,o,s]  = sum_d Wr[o,d] ref[s,i,d] + br[o]          (output 1)
    u[i,s]    = sum_o v[o] tanh(qp[i,o] + e[i,o,s])
    logits    = 10 * tanh(u)                               (output 2)

Sharding: data-parallel over batch across 8 NeuronCores (16 batches/core),
weights replicated.  No collectives needed.

Per-core design (measured ~214us on silicon, all engines 78-88% busy):
  - ref is cast f32->bf16 during the HBM->SBUF DMA (SWDGE gpsimd queue);
    the cast costs DMA-engine time but stays off every compute engine.
  - The [s,d] -> [d,s] reorientation needed for the d-contraction runs on
    the PE as 128x128 bf16 transpose-mode matmuls, staged through PSUM and
    copied to SBUF by the DVE.
  - e = WrT.T @ refT in bf16 (f32 PSUM accumulate, N=512 per bank);
    DVE adds br while copying PSUM->SBUF; e is stored bf16 (host upcasts,
    rounding ~2e-4 of scale, far under tolerance).
  - ScalarE computes t = tanh(e_psum + (qp[i]+bq+br) per-partition bias)
    straight from PSUM; PE reduces u = v.T @ t as f32r/bf16 matvecs;
    logits = 10*tanh(u) epilogue per (batch, s-chunk).
  - qp = Wq @ q.T runs in f32r, emitted lazily inside iteration (0,0) so
    the in-order PE queue is not head-of-line blocked on the Wq/q loads.
  - A 40-transpose warmup burst opens the PE HAM clock gate (1.2->2.4GHz)
    before the first real matmuls; iteration-0 ref loads are quartered so
    the transpose pipeline starts ~4us earlier.
"""

from contextlib import ExitStack

import numpy as np

import concourse.bass as bass
import concourse.bacc as bacc
import concourse.mybir as mybir
import concourse.tile as tile
from concourse import masks
from concourse.bass_utils import run_bass_kernel_spmd

F32 = mybir.dt.float32
F32R = mybir.dt.float32r
BF16 = mybir.dt.bfloat16

B, S, D = 128, 1024, 512
NCORES = 8
BL = B // NCORES          # local batches per core
C_SCALE = 10.0

P = 128                   # partitions
NDC = D // P              # d-chunks (contraction)
NOC = D // P              # o-chunks (output channels)
SC = 512                  # s-chunk (PSUM bank free size in f32)
NSC = S // SC             # s-chunks
NSS = SC // P             # 128-row subtiles per s-chunk


def build_nc() -> bass.Bass:
    nc = bacc.Bacc()

    q_p = nc.declare_dram_parameter("q", [BL, D], F32, isOutput=False)
    ref_p = nc.declare_dram_parameter("ref", [S, BL, D], F32, isOutput=False)
    wq_p = nc.declare_dram_parameter("Wq", [D, D], F32, isOutput=False)
    bq_p = nc.declare_dram_parameter("bq", [D], F32, isOutput=False)
    wr_p = nc.declare_dram_parameter("Wr", [D, D], F32, isOutput=False)
    br_p = nc.declare_dram_parameter("br", [D], F32, isOutput=False)
    v_p = nc.declare_dram_parameter("v", [D], F32, isOutput=False)
    e_p = nc.declare_dram_parameter("e", [BL, D, S], BF16, isOutput=True)
    lg_p = nc.declare_dram_parameter("logits", [BL, S], F32, isOutput=True)

    with tile.TileContext(nc) as tc, ExitStack() as ctx:
        const = ctx.enter_context(tc.tile_pool(name="const", bufs=1))
        wn_pool = ctx.enter_context(tc.tile_pool(name="wn", bufs=8))
        rin_pool = ctx.enter_context(tc.tile_pool(name="rin", bufs=8))
        rt_pool = ctx.enter_context(tc.tile_pool(name="rt", bufs=6))
        t_pool = ctx.enter_context(tc.tile_pool(name="tt", bufs=10))
        e_pool = ctx.enter_context(tc.tile_pool(name="esb", bufs=10))
        us_pool = ctx.enter_context(tc.tile_pool(name="usmall", bufs=4))
        psrt_pool = ctx.enter_context(tc.tile_pool(name="psrt", bufs=4, space="PSUM"))
        pse_pool = ctx.enter_context(tc.tile_pool(name="pse", bufs=3, space="PSUM"))
        psu_pool = ctx.enter_context(tc.tile_pool(name="psu", bufs=1, space="PSUM"))

        ident = const.tile([P, P], F32, tag="ident")
        masks.make_identity(nc, ident[:])
        ident_b = const.tile([P, P], BF16, tag="identb")
        masks.make_identity(nc, ident_b[:])

        # HAM warmup: ~40 back-to-back 128x128 transposes keep the PE busy
        # >3.4us so the clock gate opens before the real matmuls arrive.
        warm_ps = psrt_pool.tile([P, P], F32, tag="trps")
        for _ in range(40):
            nc.tensor.transpose(warm_ps[:], ident[:], ident[:])

        # ---- weights: natural load + PE transpose to [d, o] layout ----
        # Wr -> bf16 (e-matmul), Wq -> f32r (qp matmul)
        wrT = [const.tile([P, D], BF16, tag=f"wrT{dc}", name=f"wrT{dc}")
               for dc in range(NDC)]
        wqT = [const.tile([P, D], F32R, tag=f"wqT{dc}", name=f"wqT{dc}")
               for dc in range(NDC)]
        def load_wT(w_param, wT, dma_engine=None):
            eng = dma_engine or nc.sync
            wn = []
            for oc in range(NOC):
                t = wn_pool.tile([P, D], F32, tag="wn", name=f"wn{oc}")
                eng.dma_start(t[:], w_param[oc * P:(oc + 1) * P, :])
                wn.append(t)
            for dc in range(NDC):
                ps = psrt_pool.tile([P, SC], F32, tag="trps", name=f"wps{dc}")
                for oc in range(NOC):
                    nc.tensor.transpose(
                        ps[:, oc * P:(oc + 1) * P],
                        wn[oc][:, dc * P:(dc + 1) * P],
                        ident[:],
                    )
                nc.vector.tensor_copy(wT[dc][:], ps[:, :D])

        load_wT(wr_p, wrT)

        # ---- per-partition vectors: [512] -> [128, 4] column tiles ----
        brs = const.tile([P, NOC], F32, tag="brs")
        bqs = const.tile([P, NOC], F32, tag="bqs")
        v_sb = const.tile([P, NOC], F32, tag="v")
        nc.scalar.dma_start(brs[:], br_p[:].rearrange("(c p) -> p c", p=P))
        nc.scalar.dma_start(bqs[:], bq_p[:].rearrange("(c p) -> p c", p=P))
        nc.scalar.dma_start(v_sb[:], v_p[:].rearrange("(c p) -> p c", p=P))
        bqbr = const.tile([P, NOC], F32, tag="bqbr")
        nc.vector.tensor_add(bqbr[:], bqs[:], brs[:])
        v_b = const.tile([P, NOC], BF16, tag="vb")
        nc.vector.tensor_copy(v_b[:], v_sb[:])

        # ---- qp^T = Wq @ q^T  (+ bq + br), [o, i] layout ----
        # Emitted lazily inside iteration (0,0) so the in-order PE queue is
        # not blocked on the Wq/q load chain before iteration-0 work.
        qpb = [const.tile([P, BL], F32, tag=f"qpb{oc}", name=f"qpb{oc}")
               for oc in range(NOC)]

        def emit_qp():
            load_wT(wq_p, wqT, dma_engine=nc.scalar)
            q_sb = const.tile([BL, D], F32, tag="qsb")
            nc.scalar.dma_start(q_sb[:], q_p[:])
            qT = [const.tile([P, BL], F32R, tag=f"qT{dc}", name=f"qT{dc}")
                  for dc in range(NDC)]
            for dc in range(NDC):
                ps = psrt_pool.tile([P, SC], F32, tag="trps", name=f"qps{dc}")
                nc.tensor.transpose(
                    ps[:, :BL], q_sb[:, dc * P:(dc + 1) * P], ident[:BL, :BL]
                )
                nc.vector.tensor_copy(qT[dc][:], ps[:, :BL])
            for oc in range(NOC):
                ps = psrt_pool.tile([P, SC], F32, tag="trps", name=f"qpps{oc}")
                for dc in range(NDC):
                    nc.tensor.matmul(
                        ps[:, :BL],
                        wqT[dc][:, oc * P:(oc + 1) * P],
                        qT[dc][:],
                        start=(dc == 0),
                        stop=(dc == NDC - 1),
                    )
                nc.vector.tensor_scalar_add(
                    qpb[oc][:], ps[:, :BL], bqbr[:, oc:oc + 1]
                )

        # ---- main loop ----
        # The matvec/logits epilogue of iteration k is emitted after
        # iteration k+1's transposes, so the PE never stalls waiting for
        # the ACT tanh chain.
        pending_tail = None
        for i in range(BL):
            esb_i = [e_pool.tile([P, S], BF16, tag="esb", name=f"esb{i}_{oc}")
                     for oc in range(NOC)]
            for sc in range(NSC):
                s0 = sc * SC
                # one cast-DMA: ref[s0:s0+512, i, :] f32 -> bf16 [128,(ss d)]
                # (first iteration: split into quarters so the transpose
                # pipeline starts ~4us earlier during the ramp)
                rin = rin_pool.tile([P, NSS * D], BF16, tag="rin")
                ref_v = ref_p[s0:s0 + SC, i, :].rearrange(
                    "(ss p) d -> p ss d", p=P
                )
                rin_v = rin[:].rearrange("p (ss d) -> p ss d", ss=NSS)
                if i == 0:
                    for ss in range(NSS):
                        nc.gpsimd.dma_start(
                            rin_v[:, ss:ss + 1, :], ref_v[:, ss:ss + 1, :]
                        )
                else:
                    nc.gpsimd.dma_start(rin_v, ref_v)
                # PE transposes (bf16): rT[dc][p, s_l] = refT[dc*128+p, s0+s_l]
                rT = []
                for dc in range(NDC):
                    psb = psrt_pool.tile([P, SC], BF16, tag="trps")
                    for ss in range(NSS):
                        nc.tensor.transpose(
                            psb[:, ss * P:(ss + 1) * P],
                            rin[:, ss * D + dc * P:ss * D + (dc + 1) * P],
                            ident_b[:],
                        )
                    t = rt_pool.tile([P, SC], BF16, tag="rt")
                    nc.vector.tensor_copy(t[:], psb[:])
                    rT.append(t)
                if pending_tail is not None:
                    pending_tail()
                    pending_tail = None
                pss = []
                for oc in range(NOC):
                    ps = pse_pool.tile([P, SC], F32, tag="eps")
                    for dc in range(NDC):
                        nc.tensor.matmul(
                            ps[:],
                            wrT[dc][:, oc * P:(oc + 1) * P],
                            rT[dc][:],
                            start=(dc == 0),
                            stop=(dc == NDC - 1),
                        )
                    nc.vector.tensor_scalar_add(
                        esb_i[oc][:, s0:s0 + SC], ps[:], brs[:, oc:oc + 1]
                    )
                    nc.sync.dma_start(
                        e_p[i, oc * P:(oc + 1) * P, s0:s0 + SC],
                        esb_i[oc][:, s0:s0 + SC],
                    )
                    pss.append(ps)
                if i == 0 and sc == 0:
                    emit_qp()
                tsb = []
                for oc in range(NOC):
                    t = t_pool.tile([P, SC], BF16, tag="tt")
                    nc.scalar.activation(
                        t[:], pss[oc][:], mybir.ActivationFunctionType.Tanh,
                        bias=qpb[oc][:, i:i + 1],
                    )
                    tsb.append(t)
                def make_tail(tsb=tsb, i=i, s0=s0):
                    def tail():
                        psu = psu_pool.tile([1, SC], F32, tag="ups", name="psu")
                        for oc in range(NOC):
                            nc.tensor.matmul(
                                psu[:],
                                v_b[:, oc:oc + 1],
                                tsb[oc][:],
                                start=(oc == 0),
                                stop=(oc == NOC - 1),
                            )
                        ut = us_pool.tile([1, SC], F32, tag="ut", name="ut")
                        nc.scalar.activation(
                            ut[:], psu[:], mybir.ActivationFunctionType.Tanh
                        )
                        lgt = us_pool.tile([1, SC], F32, tag="lgt", name="lgt")
                        nc.scalar.activation(
                            lgt[:], ut[:], mybir.ActivationFunctionType.Copy,
                            scale=C_SCALE,
                        )
                        nc.sync.dma_start(lg_p[i, s0:s0 + SC], lgt[:])
                    return tail

                pending_tail = make_tail()


        if pending_tail is not None:
            pending_tail()

    nc.finalize()
    return nc


_NC_CACHE = None


def _get_nc() -> bass.Bass:
    global _NC_CACHE
    if _NC_CACHE is None:
        _NC_CACHE = build_nc()
    return _NC_CACHE


def _make_in_maps(q, ref, Wq, bq, Wr, br, v):
    in_maps = []
    for c in range(NCORES):
        sl = slice(c * BL, (c + 1) * BL)
        in_maps.append({
            "q": np.ascontiguousarray(q[sl], dtype=np.float32),
            "ref": np.ascontiguousarray(ref[:, sl, :], dtype=np.float32),
            "Wq": np.ascontiguousarray(Wq, dtype=np.float32),
            "bq": np.ascontiguousarray(bq, dtype=np.float32),
            "Wr": np.ascontiguousarray(Wr, dtype=np.float32),
            "br": np.ascontiguousarray(br, dtype=np.float32),
            "v": np.ascontiguousarray(v, dtype=np.float32),
        })
    return in_maps


def run_kernel(q, ref, Wq, bq, Wr, br, v, trace=False):
    """Runs on 8 NeuronCores; returns ((e, logits), BassKernelResults)."""
    nc = _get_nc()
    in_maps = _make_in_maps(q, ref, Wq, bq, Wr, br, v)
    res = run_bass_kernel_spmd(nc, in_maps, core_ids=list(range(NCORES)),
                               trace=trace)
    e = np.concatenate(
        [res.results[c]["e"].astype(np.float32) for c in range(NCORES)], axis=0
    )
    logits = np.concatenate(
        [res.results[c]["logits"] for c in range(NCORES)], axis=0
    )
    return (e, logits), res


def kernel(q, ref, Wq, bq, Wr, br, v):
    (e, logits), _ = run_kernel(q, ref, Wq, bq, Wr, br, v)
    return e, logits


# revision 31
# speedup vs baseline: 1.0247x; 1.0247x over previous
"""Trainium2 Bass kernel for nn_Attention (additive-attention scoring module).

reference math (B=128, S=1024, D=512):
    qp[i,o]   = sum_d q[i,d] Wq[o,d] + bq[o]
    e[i,o,s]  = sum_d Wr[o,d] ref[s,i,d] + br[o]          (output 1)
    u[i,s]    = sum_o v[o] tanh(qp[i,o] + e[i,o,s])
    logits    = 10 * tanh(u)                               (output 2)

Sharding: data-parallel over batch across 8 NeuronCores (16 batches/core),
weights replicated.  No collectives needed.

Per-core design (measured ~214us on silicon, all engines 78-88% busy):
  - ref is cast f32->bf16 during the HBM->SBUF DMA (SWDGE gpsimd queue);
    the cast costs DMA-engine time but stays off every compute engine.
  - The [s,d] -> [d,s] reorientation needed for the d-contraction runs on
    the PE as 128x128 bf16 transpose-mode matmuls, staged through PSUM and
    copied to SBUF by the DVE.
  - e = WrT.T @ refT in bf16 (f32 PSUM accumulate, N=512 per bank);
    DVE adds br while copying PSUM->SBUF; e is stored bf16 (host upcasts,
    rounding ~2e-4 of scale, far under tolerance).
  - ScalarE computes t = tanh(e_psum + (qp[i]+bq+br) per-partition bias)
    straight from PSUM; PE reduces u = v.T @ t as f32r/bf16 matvecs;
    logits = 10*tanh(u) epilogue per (batch, s-chunk).
  - qp = Wq @ q.T runs in f32r, emitted lazily inside iteration (0,0) so
    the in-order PE queue is not head-of-line blocked on the Wq/q loads.
  - A 40-transpose warmup burst opens the PE HAM clock gate (1.2->2.4GHz)
    before the first real matmuls; iteration-0 ref loads are quartered so
    the transpose pipeline starts ~4us earlier.
"""

from contextlib import ExitStack

import numpy as np

import concourse.bass as bass
import concourse.bacc as bacc
import concourse.mybir as mybir
import concourse.tile as tile
from concourse import masks
from concourse.bass_utils import run_bass_kernel_spmd

F32 = mybir.dt.float32
F32R = mybir.dt.float32r
BF16 = mybir.dt.bfloat16

B, S, D = 128, 1024, 512
NCORES = 8
BL = B // NCORES          # local batches per core
C_SCALE = 10.0

P = 128                   # partitions
NDC = D // P              # d-chunks (contraction)
NOC = D // P              # o-chunks (output channels)
SC = 512                  # s-chunk (PSUM bank free size in f32)
NSC = S // SC             # s-chunks
NSS = SC // P             # 128-row subtiles per s-chunk


def build_nc() -> bass.Bass:
    nc = bacc.Bacc()

    q_p = nc.declare_dram_parameter("q", [BL, D], F32, isOutput=False)
    ref_p = nc.declare_dram_parameter("ref", [S, BL, D], F32, isOutput=False)
    wq_p = nc.declare_dram_parameter("Wq", [D, D], F32, isOutput=False)
    bq_p = nc.declare_dram_parameter("bq", [D], F32, isOutput=False)
    wr_p = nc.declare_dram_parameter("Wr", [D, D], F32, isOutput=False)
    br_p = nc.declare_dram_parameter("br", [D], F32, isOutput=False)
    v_p = nc.declare_dram_parameter("v", [D], F32, isOutput=False)
    e_p = nc.declare_dram_parameter("e", [BL, D, S], BF16, isOutput=True)
    lg_p = nc.declare_dram_parameter("logits", [BL, S], F32, isOutput=True)

    with tile.TileContext(nc) as tc, ExitStack() as ctx:
        const = ctx.enter_context(tc.tile_pool(name="const", bufs=1))
        wn_pool = ctx.enter_context(tc.tile_pool(name="wn", bufs=8))
        rin_pool = ctx.enter_context(tc.tile_pool(name="rin", bufs=8))
        rt_pool = ctx.enter_context(tc.tile_pool(name="rt", bufs=6))
        t_pool = ctx.enter_context(tc.tile_pool(name="tt", bufs=10))
        e_pool = ctx.enter_context(tc.tile_pool(name="esb", bufs=10))
        us_pool = ctx.enter_context(tc.tile_pool(name="usmall", bufs=4))
        psrt_pool = ctx.enter_context(tc.tile_pool(name="psrt", bufs=4, space="PSUM"))
        pse_pool = ctx.enter_context(tc.tile_pool(name="pse", bufs=3, space="PSUM"))
        psu_pool = ctx.enter_context(tc.tile_pool(name="psu", bufs=1, space="PSUM"))

        ident = const.tile([P, P], F32, tag="ident")
        masks.make_identity(nc, ident[:])
        ident_b = const.tile([P, P], BF16, tag="identb")
        masks.make_identity(nc, ident_b[:])

        # HAM warmup: ~40 back-to-back 128x128 transposes keep the PE busy
        # >3.4us so the clock gate opens before the real matmuls arrive.
        warm_ps = psrt_pool.tile([P, P], F32, tag="trps")
        for _ in range(40):
            nc.tensor.transpose(warm_ps[:], ident[:], ident[:])

        # ---- weights: natural load + PE transpose to [d, o] layout ----
        # Wr -> bf16 (e-matmul), Wq -> f32r (qp matmul)
        wrT = [const.tile([P, D], BF16, tag=f"wrT{dc}", name=f"wrT{dc}")
               for dc in range(NDC)]
        wqT = [const.tile([P, D], F32R, tag=f"wqT{dc}", name=f"wqT{dc}")
               for dc in range(NDC)]
        def load_wT(w_param, wT, dma_engine=None):
            eng = dma_engine or nc.sync
            wn = []
            for oc in range(NOC):
                t = wn_pool.tile([P, D], F32, tag="wn", name=f"wn{oc}")
                eng.dma_start(t[:], w_param[oc * P:(oc + 1) * P, :])
                wn.append(t)
            for dc in range(NDC):
                ps = psrt_pool.tile([P, SC], F32, tag="trps", name=f"wps{dc}")
                for oc in range(NOC):
                    nc.tensor.transpose(
                        ps[:, oc * P:(oc + 1) * P],
                        wn[oc][:, dc * P:(dc + 1) * P],
                        ident[:],
                    )
                nc.vector.tensor_copy(wT[dc][:], ps[:, :D])

        load_wT(wr_p, wrT)

        # ---- per-partition vectors: [512] -> [128, 4] column tiles ----
        brs = const.tile([P, NOC], F32, tag="brs")
        bqs = const.tile([P, NOC], F32, tag="bqs")
        v_sb = const.tile([P, NOC], F32, tag="v")
        nc.scalar.dma_start(brs[:], br_p[:].rearrange("(c p) -> p c", p=P))
        nc.scalar.dma_start(bqs[:], bq_p[:].rearrange("(c p) -> p c", p=P))
        nc.scalar.dma_start(v_sb[:], v_p[:].rearrange("(c p) -> p c", p=P))
        bqbr = const.tile([P, NOC], F32, tag="bqbr")
        nc.vector.tensor_add(bqbr[:], bqs[:], brs[:])
        v_b = const.tile([P, NOC], BF16, tag="vb")
        nc.vector.tensor_copy(v_b[:], v_sb[:])

        # ---- qp^T = Wq @ q^T  (+ bq + br), [o, i] layout ----
        # Emitted lazily inside iteration (0,0) so the in-order PE queue is
        # not blocked on the Wq/q load chain before iteration-0 work.
        qpb = [const.tile([P, BL], F32, tag=f"qpb{oc}", name=f"qpb{oc}")
               for oc in range(NOC)]

        def emit_qp():
            load_wT(wq_p, wqT, dma_engine=nc.scalar)
            q_sb = const.tile([BL, D], F32, tag="qsb")
            nc.scalar.dma_start(q_sb[:], q_p[:])
            qT = [const.tile([P, BL], F32R, tag=f"qT{dc}", name=f"qT{dc}")
                  for dc in range(NDC)]
            for dc in range(NDC):
                ps = psrt_pool.tile([P, SC], F32, tag="trps", name=f"qps{dc}")
                nc.tensor.transpose(
                    ps[:, :BL], q_sb[:, dc * P:(dc + 1) * P], ident[:BL, :BL]
                )
                nc.vector.tensor_copy(qT[dc][:], ps[:, :BL])
            for oc in range(NOC):
                ps = psrt_pool.tile([P, SC], F32, tag="trps", name=f"qpps{oc}")
                for dc in range(NDC):
                    nc.tensor.matmul(
                        ps[:, :BL],
                        wqT[dc][:, oc * P:(oc + 1) * P],
                        qT[dc][:],
                        start=(dc == 0),
                        stop=(dc == NDC - 1),
                    )
                nc.vector.tensor_scalar_add(
                    qpb[oc][:], ps[:, :BL], bqbr[:, oc:oc + 1]
                )

        # ---- main loop ----
        for i in range(BL):
            esb_i = [e_pool.tile([P, S], BF16, tag="esb", name=f"esb{i}_{oc}")
                     for oc in range(NOC)]
            for sc in range(NSC):
                s0 = sc * SC
                # one cast-DMA: ref[s0:s0+512, i, :] f32 -> bf16 [128,(ss d)]
                # (first iteration: split into quarters so the transpose
                # pipeline starts ~4us earlier during the ramp)
                rin = rin_pool.tile([P, NSS * D], BF16, tag="rin")
                ref_v = ref_p[s0:s0 + SC, i, :].rearrange(
                    "(ss p) d -> p ss d", p=P
                )
                rin_v = rin[:].rearrange("p (ss d) -> p ss d", ss=NSS)
                if i == 0:
                    for ss in range(NSS):
                        nc.gpsimd.dma_start(
                            rin_v[:, ss:ss + 1, :], ref_v[:, ss:ss + 1, :]
                        )
                else:
                    nc.gpsimd.dma_start(rin_v, ref_v)
                # PE transposes (bf16): rT[dc][p, s_l] = refT[dc*128+p, s0+s_l]
                rT = []
                for dc in range(NDC):
                    psb = psrt_pool.tile([P, SC], BF16, tag="trps")
                    for ss in range(NSS):
                        nc.tensor.transpose(
                            psb[:, ss * P:(ss + 1) * P],
                            rin[:, ss * D + dc * P:ss * D + (dc + 1) * P],
                            ident_b[:],
                        )
                    t = rt_pool.tile([P, SC], BF16, tag="rt")
                    nc.vector.tensor_copy(t[:], psb[:])
                    rT.append(t)
                pss = []
                for oc in range(NOC):
                    ps = pse_pool.tile([P, SC], F32, tag="eps")
                    for dc in range(NDC):
                        nc.tensor.matmul(
                            ps[:],
                            wrT[dc][:, oc * P:(oc + 1) * P],
                            rT[dc][:],
                            start=(dc == 0),
                            stop=(dc == NDC - 1),
                        )
                    nc.vector.tensor_scalar_add(
                        esb_i[oc][:, s0:s0 + SC], ps[:], brs[:, oc:oc + 1]
                    )
                    nc.sync.dma_start(
                        e_p[i, oc * P:(oc + 1) * P, s0:s0 + SC],
                        esb_i[oc][:, s0:s0 + SC],
                    )
                    pss.append(ps)
                if i == 0 and sc == 0:
                    emit_qp()
                tsb = []
                for oc in range(NOC):
                    t = t_pool.tile([P, SC], BF16, tag="tt")
                    nc.scalar.activation(
                        t[:], pss[oc][:], mybir.ActivationFunctionType.Tanh,
                        bias=qpb[oc][:, i:i + 1],
                    )
                    tsb.append(t)
                psu = psu_pool.tile([1, SC], F32, tag="ups")
                for oc in range(NOC):
                    nc.tensor.matmul(
                        psu[:],
                        v_b[:, oc:oc + 1],
                        tsb[oc][:],
                        start=(oc == 0),
                        stop=(oc == NOC - 1),
                    )
                ut = us_pool.tile([1, SC], F32, tag="ut")
                nc.scalar.activation(
                    ut[:], psu[:], mybir.ActivationFunctionType.Tanh
                )
                lgt = us_pool.tile([1, SC], F32, tag="lgt")
                nc.scalar.activation(
                    lgt[:], ut[:], mybir.ActivationFunctionType.Copy,
                    scale=C_SCALE,
                )
                nc.sync.dma_start(lg_p[i, s0:s0 + SC], lgt[:])


    nc.finalize()
    return nc


_NC_CACHE = None


def _get_nc() -> bass.Bass:
    global _NC_CACHE
    if _NC_CACHE is None:
        _NC_CACHE = build_nc()
    return _NC_CACHE


def _make_in_maps(q, ref, Wq, bq, Wr, br, v):
    in_maps = []
    for c in range(NCORES):
        sl = slice(c * BL, (c + 1) * BL)
        in_maps.append({
            "q": np.ascontiguousarray(q[sl], dtype=np.float32),
            "ref": np.ascontiguousarray(ref[:, sl, :], dtype=np.float32),
            "Wq": np.ascontiguousarray(Wq, dtype=np.float32),
            "bq": np.ascontiguousarray(bq, dtype=np.float32),
            "Wr": np.ascontiguousarray(Wr, dtype=np.float32),
            "br": np.ascontiguousarray(br, dtype=np.float32),
            "v": np.ascontiguousarray(v, dtype=np.float32),
        })
    return in_maps


def run_kernel(q, ref, Wq, bq, Wr, br, v, trace=False):
    """Runs on 8 NeuronCores; returns ((e, logits), BassKernelResults)."""
    nc = _get_nc()
    in_maps = _make_in_maps(q, ref, Wq, bq, Wr, br, v)
    res = run_bass_kernel_spmd(nc, in_maps, core_ids=list(range(NCORES)),
                               trace=trace)
    e = np.concatenate(
        [res.results[c]["e"].astype(np.float32) for c in range(NCORES)], axis=0
    )
    logits = np.concatenate(
        [res.results[c]["logits"] for c in range(NCORES)], axis=0
    )
    return (e, logits), res


def kernel(q, ref, Wq, bq, Wr, br, v):
    (e, logits), _ = run_kernel(q, ref, Wq, bq, Wr, br, v)
    return e, logits
